# revision 16
# baseline (speedup 1.0000x reference)
"""Trainium2 Bass kernel for nn_DeformableInception.

Architecture (per core, one batch element; batch-parallel over 8 cores):
  1. Host prep: gather indices + bilinear corner weights from deform maps;
     int8 (scale 32) padded vertical-pair image so one 1KB gather descriptor
     fetches all 4 bilinear corners x 256 channels.
  2. dma_gather (SWDGE, 4 queues round-robin): positions-on-partitions
     corner blocks, int8.
  3. Bilinear blend split across THREE engine streams (DVE / Pool / ACT
     scale-ops), emitted in 8-group waves so dependent ops sit 8 apart in
     each engine queue (no back-to-back semaphore stalls).
  4. Transpose S^T -> S via PE identity matmuls (f16 PSUM) with one wide
     ACT evacuation per (br,kk,ch), emitted per blend block to keep the
     PE stream dense (p-state) and overlapped.
  5. Branch einsum per br half-round: W' [ck,o-block] stationary x S
     [ck,pos512] -> cat PSUM fp32; evac + 1x1 + h-store + 3x3 run as a
     software-pipelined tail one round behind (keeps ACT from stalling on
     PE results).
  6. 3x3 conv via shifted free-dim APs over a zero-padded h grid, output
     rows DMA'd out as they complete.
All matmuls fp16 operands with fp32 PSUM accumulation.
"""
import sys
import numpy as np

sys.path.insert(0, '/opt/trn_rl_repo')

import bass_rust
import concourse.bacc as bacc
import concourse.bass as bass
import concourse.mybir as mybir
from concourse.tile import TileContext

F16 = mybir.dt.float16
F32 = mybir.dt.float32
I16 = mybir.dt.int16
I8 = mybir.dt.int8
AF = mybir.ActivationFunctionType
ALU = mybir.AluOpType

C = 256          # input channels
O = 256          # per-branch output channels
KK = 9           # 3x3 taps
NCLS = 324
G2 = 512         # cat channels
CKT = 18         # branch contraction tiles (9 taps x 2 c-halves)
CFT = 36         # 3x3 contraction tiles (9 taps x 4 ic-tiles)
XS = 32.0        # int8 quantization scale for x

# blend stream split per round (72 groups of (br,kk,gi)). The GPSIMD (Pool)
# engine only implements tensor-tensor ALU ops (no scalar-ptr, no PSUM), so
# it contributes the pair-combine adds; ACT does scale-ops; DVE does stt.
#   dve3p: 2 ACT scales + 2 DVE stt + Pool TT-add
#   dve3:  2 ACT scales + 2 DVE stt + DVE TT-add
#   dve2:  DVE scale + 3 DVE stt          (fully DVE-resident)
N_3P = 64
N_3 = 4
BLOCK = 8        # groups per emission block (dependent ops 8 apart)
VBUFS = 6        # gather tile double-buffering depth


def _corner_geom(dm, Hd):
    """y0, x0 (int), corner weights [4,KK,H,W] for one deform map [18,H,W]."""
    Wd = Hd
    off = dm.reshape(KK, 2, Hd, Wd)
    dy, dx = off[:, 0], off[:, 1]
    ky = np.repeat(np.arange(3), 3).astype(np.float32)
    kx = np.tile(np.arange(3), 3).astype(np.float32)
    py = np.arange(Hd, dtype=np.float32)[None, :, None] + (ky - 1)[:, None, None] + dy
    px = np.arange(Wd, dtype=np.float32)[None, None, :] + (kx - 1)[:, None, None] + dx
    y0 = np.floor(py)
    x0 = np.floor(px)
    fy = (py - y0).astype(np.float32)
    fx = (px - x0).astype(np.float32)
    w00 = (1 - fy) * (1 - fx)
    w10 = fy * (1 - fx)
    w01 = (1 - fy) * fx
    w11 = fy * fx
    return (y0.astype(np.int64), x0.astype(np.int64),
            np.stack([w00, w10, w01, w11], 0))


# ---------------------------------------------------------------- host prep
def host_prep(x, dm0, dm1, w_dc0, w_dc1, w_cc, b_cc, w_f, b_f, Hd, P):
    """Per-core input prep. x: [C,Hd,Hd] fp32. P: global pad. Returns dict."""
    Wd = Hd
    NPOS = Hd * Wd
    NG = NPOS // 128
    NR = NG // 4

    geos = [_corner_geom(dm0, Hd), _corner_geom(dm1, Hd)]
    H2 = Hd + 2 * P
    W2 = Wd + 2 * P
    R = H2 * W2
    assert R <= 32766, f"pad too large: P={P}"

    # padded image, int8 (scale XS), HWC; one extra row so row pairs exist
    xq = np.clip(np.rint(np.asarray(x, np.float32) * XS), -127, 127).astype(np.int8)
    xp = np.zeros((H2 + 1, W2, C), np.int8)
    xp[P:P + Hd, P:P + Wd, :] = np.transpose(xq, (1, 2, 0))
    x2 = np.concatenate([xp[:H2], xp[1:H2 + 1]], axis=2).reshape(R, 2 * C)

    # indices: clip fully-OOB cases into the zero border (contributions are 0)
    idx_cols = 2 * KK * NR * 32
    idx_sb = np.zeros((128, idx_cols), np.int16)
    wts = np.zeros((128, 2 * KK * 4 * NG), np.float32)
    for br in range(2):
        y0, x0, w4 = geos[br]
        y0c = np.clip(y0, -P, Hd - 1 + P)
        x0c = np.clip(x0, -P, Wd - 2 + P)
        ridx = ((y0c + P) * W2 + (x0c + P)).astype(np.int64)
        assert ridx.min() >= 0 and ridx.max() <= R - 2
        rflat = ridx.reshape(KK, NPOS)
        wflat = (w4 / XS).reshape(4, KK, NPOS)      # dequant folded into weights
        for kk in range(KK):
            for r in range(NR):
                chunk = rflat[kk, r * 512:(r + 1) * 512].astype(np.int16)
                wrap = chunk.reshape(32, 16).T               # [16,32] col-major
                col0 = (br * KK + kk) * (NR * 32) + r * 32
                idx_sb[:, col0:col0 + 32] = np.tile(wrap, (8, 1))
            for cr in range(4):
                cols = wflat[cr, kk].reshape(NG, 128).T      # [128, NG]
                col0 = ((br * KK + kk) * 4 + cr) * NG
                wts[:, col0:col0 + NG] = cols

    # branch weights W': [2*18, 128, 256] fp16  (ck tile = kk*2 + chalf)
    wp = np.zeros((2, CKT, 128, O), np.float16)
    for br, wdc in enumerate((w_dc0, w_dc1)):
        w3 = wdc.reshape(O, C, KK)                           # [o, c, kk]
        for kk in range(KK):
            for ch in range(2):
                blk = w3[:, ch * 128:(ch + 1) * 128, kk]     # [o, 128]
                wp[br, kk * 2 + ch] = blk.T.astype(np.float16)

    # 1x1 weights: [4, 128, 512] fp16
    wcc = np.zeros((4, 128, G2), np.float16)
    for ic in range(4):
        wcc[ic] = w_cc[:, ic * 128:(ic + 1) * 128, 0, 0].T.astype(np.float16)

    # 3x3 weights: [36, 128, 324] fp16 (tile t = tap*4 + ic_tile)
    wf = np.zeros((CFT, 128, NCLS), np.float16)
    for tap in range(KK):
        for ic in range(4):
            blk = w_f[:, ic * 128:(ic + 1) * 128, tap // 3, tap % 3]
            wf[tap * 4 + ic] = blk.T.astype(np.float16)

    bcc = np.zeros((128, 4), np.float32)
    for ic in range(4):
        bcc[:, ic] = b_cc[ic * 128:(ic + 1) * 128]
    bf = np.zeros((128, 3), np.float32)
    bf_pad = np.zeros(384, np.float32)
    bf_pad[:NCLS] = b_f
    for ot in range(3):
        bf[:, ot] = bf_pad[ot * 128:(ot + 1) * 128]

    return {
        'x2': x2, 'idx': idx_sb, 'wts': wts, 'wp': wp.reshape(2 * CKT, 128, O),
        'wcc': wcc, 'wf': wf, 'bcc': bcc, 'bf': bf,
        'ident': np.eye(128, dtype=np.float16),
    }


def _spread(counts):
    """Evenly interleave class labels; counts: dict label -> count."""
    total = sum(counts.values())
    acc = dict.fromkeys(counts, 0.0)
    out = []
    for i in range(1, total + 1):
        k = max(counts, key=lambda k: counts[k] * i / total - acc[k])
        acc[k] += 1.0
        out.append(k)
    return out


# ------------------------------------------------------------- kernel build
def build_kernel(Hd, R, mode='full', reps=1, n_3p=N_3P, n_3=N_3):
    """Build the Bacc kernel for image size Hd (R = padded x2 rows).
    mode: 'full' | 'gatheronly' | 'noblend' | 'notr2' | 'noconv3'.
    reps: repeat the whole pipeline (for marginal-cost timing)."""
    Wd = Hd
    NPOS = Hd * Wd
    NG = NPOS // 128
    NR = NG // 4          # rounds of 512 positions
    H3 = Hd + 2
    N3 = H3 * H3
    RT3 = min(H3, 512 // H3)          # padded rows per 3x3 n-tile
    NT3 = (H3 + RT3 - 1) // RT3
    RPR = 512 // Wd       # image rows per round

    nc = bacc.Bacc(None, target_bir_lowering=False, num_swdge_queues=4)

    x2_d = nc.dram_tensor('x2', [R, 2 * C], I8, kind='ExternalInput')
    idx_d = nc.dram_tensor('idx', [128, 2 * KK * NR * 32], I16, kind='ExternalInput')
    wts_d = nc.dram_tensor('wts', [128, 2 * KK * 4 * NG], F32, kind='ExternalInput')
    wp_d = nc.dram_tensor('wp', [2 * CKT, 128, O], F16, kind='ExternalInput')
    wcc_d = nc.dram_tensor('wcc', [4, 128, G2], F16, kind='ExternalInput')
    wf_d = nc.dram_tensor('wf', [CFT, 128, NCLS], F16, kind='ExternalInput')
    bcc_d = nc.dram_tensor('bcc', [128, 4], F32, kind='ExternalInput')
    bf_d = nc.dram_tensor('bf', [128, 3], F32, kind='ExternalInput')
    id_d = nc.dram_tensor('ident', [128, 128], F16, kind='ExternalInput')
    out_d = nc.dram_tensor('out', [NCLS, NPOS], F32, kind='ExternalOutput')
    dbg_d = None
    if mode == 'gatheronly':
        dbg_d = nc.dram_tensor('dbg', [128, NR * 2 * KK * 64], I8,
                               kind='ExternalOutput')

    # overlapping-window AP over x2: [R-1 rows, 1024 i8] stepping one row (512)
    win = x2_d[:, :].copy()
    win.ap = bass_rust.VecI64Pair([[2 * C, R - 1], [1, 4 * C]])

    # blend stream pattern over the 72 (br,kk,gi) groups of a round
    n_2 = 72 - n_3p - n_3
    pattern = _spread({'dve3p': n_3p, 'dve3': n_3, 'dve2': n_2})

    with TileContext(nc) as tc:
        with tc.tile_pool(name='const', bufs=1) as cpool, \
             tc.tile_pool(name='vg', bufs=VBUFS) as vpool, \
             tc.tile_pool(name='st', bufs=3) as stpool, \
             tc.tile_pool(name='sasm', bufs=1) as sapool, \
             tc.tile_pool(name='cat', bufs=2) as catpool, \
             tc.tile_pool(name='hbuf', bufs=1) as hpool, \
             tc.tile_pool(name='outs', bufs=2) as opool, \
             tc.tile_pool(name='ptr', bufs=2, space='PSUM') as trppool, \
             tc.tile_pool(name='pcat', bufs=3, space='PSUM') as catppool, \
             tc.tile_pool(name='ph', bufs=1, space='PSUM') as hppool, \
             tc.tile_pool(name='pf', bufs=2, space='PSUM') as fppool:

            # ---- constants ----
            idx_t = cpool.tile([128, 2 * KK * NR * 32], I16, tag='idx')
            nc.sync.dma_start(idx_t[:], idx_d[:])
            wts_t = cpool.tile([128, 2 * KK * 4 * NG], F32, tag='wts')
            nc.sync.dma_start(wts_t[:], wts_d[:])
            ident = cpool.tile([128, 128], F16, tag='ident')
            nc.sync.dma_start(ident[:], id_d[:])
            wp_t = []
            for i in range(2 * CKT):
                t = cpool.tile([128, O], F16, tag=f'wp{i}')
                nc.sync.dma_start(t[:], wp_d[i])
                wp_t.append(t)
            wcc_t = []
            for ic in range(4):
                t = cpool.tile([128, G2], F16, tag=f'wcc{ic}')
                nc.sync.dma_start(t[:], wcc_d[ic])
                wcc_t.append(t)
            bcc_t = cpool.tile([128, 4], F32, tag='bcc')
            nc.sync.dma_start(bcc_t[:], bcc_d[:])
            bf_t = cpool.tile([128, 3], F32, tag='bf')
            nc.sync.dma_start(bf_t[:], bf_d[:])
            wf_t = []
            for i in range(CFT):
                t = cpool.tile([128, NCLS], F16, tag=f'wf{i}')
                nc.sync.dma_start(t[:], wf_d[i])
                wf_t.append(t)

            # ---- padded h grid (zeroed; guard margins for 3x3 shifts) ----
            h_t = []
            for ic in range(4):
                t = hpool.tile([128, N3 + 136], F16, tag=f'h{ic}')
                nc.vector.memset(t[:], 0.0)
                h_t.append(t)

            def wcol(br, kk, cr, g):
                return ((br * KK + kk) * 4 + cr) * NG + g

            OT3 = [(0, 128), (128, 128), (256, 68)]

            def emit_3x3(nt, rep):
                r0 = nt * RT3
                nrows = min(RT3, H3 - r0)
                nsz = nrows * H3
                n0 = r0 * H3
                for o, (obase, orows) in enumerate(OT3):
                    pf = fppool.tile([128, 512], F32, tag='pf',
                                     name=f'pf{rep}_{nt}_{o}')
                    for j in range(CFT):
                        tap, ic = j // 4, j % 4
                        ky, kx = tap // 3, tap % 3
                        off = (ky - 1) * H3 + (kx - 1)
                        nc.tensor.matmul(
                            pf[:orows, :nsz],
                            wf_t[j][:, obase:obase + orows],
                            h_t[ic][:, 68 + off + n0: 68 + off + n0 + nsz],
                            start=(j == 0), stop=(j == CFT - 1))
                    stg = opool.tile([128, 512], F32, tag='stg',
                                     name=f'stg{rep}_{nt}_{o}')
                    nc.scalar.activation(stg[:orows, :nsz], pf[:orows, :nsz],
                                         AF.Identity, bias=bf_t[:orows, o:o + 1])
                    vr0 = max(1, r0)
                    vr1 = min(H3 - 2, r0 + nrows - 1)
                    nvr = vr1 - vr0 + 1
                    if nvr <= 0:
                        continue
                    src2 = stg[:, :].copy()
                    pstep = src2.ap[0][0]
                    src2.offset = src2.offset + (vr0 - r0) * H3 + 1
                    src2.ap = bass_rust.VecI64Pair(
                        [[pstep, orows], [H3, nvr], [1, Wd]])
                    nc.sync.dma_start(
                        out_d[obase:obase + orows,
                              (vr0 - 1) * Wd:(vr0 - 1 + nvr) * Wd], src2)

            # ---- per-round emission helpers ----
            def emit_blend_block(block, vtiles, sasm, r, pending_evacs):
                """block: list of (slot, (br,kk,gi), kind). 5 waves, then PE
                transposes into one [128,1024] f16 PSUM tile per (br,kk).
                Dependent ops sit len(block) apart per engine queue.
                pending_evacs: DVE evac closures from the previous block,
                emitted between waves so DVE never waits on fresh PE output."""
                tiles = {}
                for slot, (br, kk, gi), kind in block:
                    stc = stpool.tile([128, C], F16, tag=f'st{slot}',
                                      name=f'st{slot}_{r}_{br}_{kk}_{gi}')
                    t2 = None
                    if kind in ('dve3', 'dve3p'):
                        t2 = stpool.tile([128, C], F16, tag=f'tm{slot}',
                                         name=f'tm{slot}_{r}_{br}_{kk}_{gi}')
                    tiles[slot] = (stc, t2)

                def w(br, kk, cr, gi):
                    g = r * 4 + gi
                    cl = wcol(br, kk, cr, g)
                    return wts_t[:, cl:cl + 1]

                # wave 1: scale corner0 -> stc (ACT; DVE for dve2)
                for slot, (br, kk, gi), kind in block:
                    stc, _ = tiles[slot]
                    v = vtiles[(br, kk)]
                    if kind == 'dve2':
                        nc.vector.tensor_scalar_mul(stc[:], v[:, gi, 0:C],
                                                    w(br, kk, 0, gi))
                    else:
                        nc.scalar.activation(stc[:], v[:, gi, 0:C], AF.Copy,
                                             scale=w(br, kk, 0, gi))
                # wave 2 (dve3/dve3p): ACT scale corner2 -> t2
                for slot, (br, kk, gi), kind in block:
                    if kind == 'dve2':
                        continue
                    _, t2 = tiles[slot]
                    v = vtiles[(br, kk)]
                    nc.scalar.activation(t2[:], v[:, gi, 2 * C:3 * C], AF.Copy,
                                         scale=w(br, kk, 2, gi))
                # wave 3: corner1 += into stc (DVE)
                for slot, (br, kk, gi), kind in block:
                    stc, _ = tiles[slot]
                    v = vtiles[(br, kk)]
                    nc.vector.scalar_tensor_tensor(
                        stc[:], v[:, gi, C:2 * C], w(br, kk, 1, gi), stc[:],
                        ALU.mult, ALU.add)
                # previous block's sa evacs: PE transposes are long done
                for ev in pending_evacs:
                    ev()
                pending_evacs.clear()
                # wave 4: corner3 -> t2 (dve3*) or corner2 -> stc (dve2), DVE
                for slot, (br, kk, gi), kind in block:
                    stc, t2 = tiles[slot]
                    v = vtiles[(br, kk)]
                    if kind in ('dve3', 'dve3p'):
                        nc.vector.scalar_tensor_tensor(
                            t2[:], v[:, gi, 3 * C:4 * C], w(br, kk, 3, gi),
                            t2[:], ALU.mult, ALU.add)
                    else:
                        nc.vector.scalar_tensor_tensor(
                            stc[:], v[:, gi, 2 * C:3 * C], w(br, kk, 2, gi),
                            stc[:], ALU.mult, ALU.add)
                # wave 5: combine (Pool TT for dve3p, DVE TT for dve3,
                # DVE stt corner3 for dve2)
                for slot, (br, kk, gi), kind in block:
                    stc, t2 = tiles[slot]
                    v = vtiles[(br, kk)]
                    if kind == 'dve3p':
                        nc.gpsimd.tensor_tensor(stc[:], stc[:], t2[:], ALU.add)
                    elif kind == 'dve3':
                        nc.vector.tensor_tensor(stc[:], stc[:], t2[:], ALU.add)
                    else:
                        nc.vector.scalar_tensor_tensor(
                            stc[:], v[:, gi, 3 * C:4 * C], w(br, kk, 3, gi),
                            stc[:], ALU.mult, ALU.add)
                # transposes: per completed (br,kk): 8 gi/ch-blocks through PE
                # (identity matmul) into one [128,1024] f16 PSUM tile; DVE
                # evacuates it (2x-mode copy) next block
                if mode == 'notr2':
                    return
                stc_of = {}
                for slot, g, kind in block:
                    stc_of[g] = tiles[slot][0]
                bks = sorted({(br, kk) for _, (br, kk, _), _ in block})
                for br, kk in bks:
                    ptr = trppool.tile([128, 1024], F16, tag='ptr',
                                       name=f'ptr{r}_{br}_{kk}')
                    for ch in range(2):
                        for gi in range(4):
                            nc.tensor.transpose(
                                ptr[:, ch * 512 + gi * 128:
                                    ch * 512 + (gi + 1) * 128],
                                stc_of[(br, kk, gi)][:, ch * 128:(ch + 1) * 128],
                                ident[:])
                    sa = sasm[(br, kk)]
                    pending_evacs.append(
                        lambda sa=sa, ptr=ptr: nc.vector.tensor_copy(sa[:], ptr[:]))

            def emit_einsum_br(br, sasm, pc_tiles, r):
                for o in range(2):
                    pc = catppool.tile([128, 512], F32, tag='pcat',
                                       name=f'pc{r}_{br}_{o}')
                    pc_tiles[(br, o)] = pc
                    for ck in range(CKT):
                        kk, ch = ck // 2, ck % 2
                        nc.tensor.matmul(
                            pc[:],
                            wp_t[br * CKT + ck][:, o * 128:(o + 1) * 128],
                            sasm[(br, kk)][:, ch * 512:(ch + 1) * 512],
                            start=(ck == 0), stop=(ck == CKT - 1))

            def emit_evac_br(br, pc_tiles, cat_tiles):
                for o in range(2):
                    ic = br * 2 + o
                    nc.scalar.activation(cat_tiles[ic][:], pc_tiles[(br, o)][:],
                                         AF.Copy)

            def make_tail(r, rep_r, cat_tiles, pc_tiles):
                """Tail closure for round r: evac br1, 1x1, h stores, 3x3."""
                rep = rep_r // NR

                def tail():
                    emit_evac_br(1, pc_tiles, cat_tiles)
                    for o in range(4):
                        ph = hppool.tile([128, 512], F32, tag='ph',
                                         name=f'ph{rep_r}_{o}')
                        for ic in range(4):
                            nc.tensor.matmul(
                                ph[:], wcc_t[ic][:, o * 128:(o + 1) * 128],
                                cat_tiles[ic][:], start=(ic == 0), stop=(ic == 3))
                        dst = h_t[o][:, :].copy()
                        pstep = dst.ap[0][0]
                        dst.offset = dst.offset + 68 + (r * RPR + 1) * H3 + 1
                        dst.ap = bass_rust.VecI64Pair(
                            [[pstep, 128], [H3, RPR], [1, Wd]])
                        nc.scalar.activation(dst, ph[:], AF.Identity,
                                             bias=bcc_t[:, o:o + 1])
                    if mode == 'noconv3':
                        return
                    ready = (r + 1) * RPR
                    for nt in range(NT3):
                        last = min(nt * RT3 + min(RT3, H3 - nt * RT3), H3 - 2)
                        prev_ready = r * RPR if r > 0 else -1
                        if last <= ready and not (last <= prev_ready):
                            emit_3x3(nt, rep)
                return tail

            # ---- main loop over rounds of 512 positions ----
            gidx = 0          # global gather counter: queue = gidx % 4 stays
            # aligned with Tile's DMASW lane round-robin (lane = gidx % 8),
            # so each sem lane only ever sees one SWDGE queue.
            pending_tail = None
            for rep_r in range(reps * NR):
                r = rep_r % NR
                vtiles = {}
                for br in range(2):
                    for kk in range(KK):
                        v = vpool.tile([128, 4, 4 * C], I8, tag='v',
                                       name=f'v{rep_r}_{br}_{kk}')
                        col0 = (br * KK + kk) * (NR * 32) + r * 32
                        nc.gpsimd.dma_gather(
                            v[:], win, idx_t[:, col0:col0 + 32],
                            512, 512, 4 * C, elem_step=2 * C,
                            queue_num=gidx % 4)
                        gidx += 1
                        vtiles[(br, kk)] = v
                if mode == 'gatheronly':
                    for br in range(2):
                        for kk in range(KK):
                            col = ((r * 2 + br) * KK + kk) * 64
                            nc.sync.dma_start(
                                dbg_d[:, col:col + 64],
                                vtiles[(br, kk)][:, 0, 0:64])
                    continue
                if mode == 'noblend':
                    continue

                # group plan for this round
                groups = [(br, kk, gi)
                          for br in range(2) for kk in range(KK)
                          for gi in range(4)]
                sasm = {}
                if mode not in ('notr2',):
                    for br in range(2):
                        for kk in range(KK):
                            sasm[(br, kk)] = sapool.tile(
                                [128, 1024], F16, tag=f'sa{br}_{kk}',
                                name=f'sa{br}_{kk}_{rep_r}')

                cat_tiles = {}
                pc_tiles = {}
                for ic in range(4):
                    cat_tiles[ic] = catpool.tile([128, 512], F16, tag=f'cat{ic}',
                                                 name=f'cat{ic}_{rep_r}')

                # per-branch: blend blocks then einsum; pending tail from the
                # previous round flushes after the 2nd block of br0, and the
                # br0 cat evac lands after the 2nd block of br1.
                blk_count = 0
                pending_evacs = []
                for br in range(2):
                    gset = [g for g in groups if g[0] == br]
                    blocks = [gset[i:i + BLOCK] for i in range(0, len(gset), BLOCK)]
                    for bi, blk in enumerate(blocks):
                        block = []
                        for slot, g in enumerate(blk):
                            gidx72 = (g[0] * KK + g[1]) * 4 + g[2]
                            block.append((slot, g, pattern[gidx72]))
                        emit_blend_block(block, vtiles, sasm, r, pending_evacs)
                        blk_count += 1
                        if blk_count == 2 and pending_tail is not None:
                            pending_tail()
                            pending_tail = None
                        if br == 1 and bi == 1 and mode not in ('notr2',):
                            emit_evac_br(0, pc_tiles, cat_tiles)
                    if mode in ('notr2',):
                        continue
                    for ev in pending_evacs:
                        ev()
                    pending_evacs.clear()
                    emit_einsum_br(br, sasm, pc_tiles, r)

                if mode in ('notr2',):
                    continue
                pending_tail = make_tail(r, rep_r, cat_tiles, pc_tiles)

            if pending_tail is not None:
                pending_tail()
                pending_tail = None

    nc.compile()
    return nc


# ----------------------------------------------------------------- driver
_CACHE = {}


def _get_kernel(Hd, R):
    key = (Hd, R)
    if key not in _CACHE:
        _CACHE[key] = build_kernel(Hd, R)
    return _CACHE[key]


def global_pad(deform_map0, deform_map1, Hd):
    """Common pad P across the whole batch (all cores share one NEFF)."""
    P = 2
    for dms in (deform_map0, deform_map1):
        for b in range(dms.shape[0]):
            y0, x0, _ = _corner_geom(np.asarray(dms[b], np.float32), Hd)
            P = max(P, int(-y0.min()), int(y0.max() - 62),
                    int(-x0.min()), int(x0.max() - 62))
    return P


def prep_all(x, deform_map0, deform_map1, w_dc0, w_dc1, w_cc, b_cc, w_f, b_f):
    x = np.asarray(x, np.float32)
    Hd = x.shape[2]
    P = global_pad(np.asarray(deform_map0, np.float32),
                   np.asarray(deform_map1, np.float32), Hd)
    in_maps = []
    for b in range(x.shape[0]):
        m = host_prep(x[b], np.asarray(deform_map0[b], np.float32),
                      np.asarray(deform_map1[b], np.float32),
                      np.asarray(w_dc0, np.float32), np.asarray(w_dc1, np.float32),
                      np.asarray(w_cc, np.float32), np.asarray(b_cc, np.float32),
                      np.asarray(w_f, np.float32), np.asarray(b_f, np.float32),
                      Hd, P)
        in_maps.append(m)
    R = in_maps[0]['x2'].shape[0]
    return in_maps, Hd, R


def kernel(x, deform_map0, deform_map1, w_dc0, w_dc1, w_cc, b_cc, w_f, b_f):
    from concourse.bass_utils import run_bass_kernel_spmd
    in_maps, Hd, R = prep_all(x, deform_map0, deform_map1, w_dc0, w_dc1,
                              w_cc, b_cc, w_f, b_f)
    B = len(in_maps)
    nc = _get_kernel(Hd, R)
    res = run_bass_kernel_spmd(nc, in_maps, core_ids=list(range(B)))
    out = np.stack([res.results[b]['out'].reshape(NCLS, Hd, Hd) for b in range(B)])
    return out.astype(np.float32)


# revision 48
# speedup vs baseline: 3.3039x; 3.3039x over previous
"""Trainium2 Bass kernel for nn_DeformableInception.

Architecture (per core, one batch element; batch-parallel over 8 cores):
  1. Host prep: gather indices + bilinear corner weights from deform maps;
     f16 padded vertical-pair image so one 2KB gather descriptor fetches
     all 4 bilinear corners x 256 channels.
  2. dma_gather (SWDGE, 4 queues round-robin): positions-on-partitions
     corner blocks, f16.
  3. Bilinear blend split across THREE engine streams (DVE / Pool / ACT
     scale-ops), emitted in 8-group waves so dependent ops sit 8 apart in
     each engine queue (no back-to-back semaphore stalls).
  4. Transpose S^T -> S via PE identity matmuls (f16 PSUM) with one wide
     ACT evacuation per (br,kk,ch), emitted per blend block to keep the
     PE stream dense (p-state) and overlapped.
  5. Branch einsum per br half-round: W' [ck,o-block] stationary x S
     [ck,pos512] -> cat PSUM fp32; evac + 1x1 + h-store + 3x3 run as a
     software-pipelined tail one round behind (keeps ACT from stalling on
     PE results).
  6. 3x3 conv via shifted free-dim APs over a zero-padded h grid, output
     rows DMA'd out as they complete.
All matmuls fp16 operands with fp32 PSUM accumulation.
"""
import sys
import numpy as np

sys.path.insert(0, '/opt/trn_rl_repo')

import bass_rust
import concourse.bacc as bacc
import concourse.bass as bass
import concourse.mybir as mybir
from concourse.tile import TileContext
from concourse import dve_ops as _dops
from concourse.dve_spec import (
    Spec as _Spec, Src0 as _Src0, Src1 as _Src1, C0 as _C0, C1 as _C1,
    lower as _dve_lower,
)
from concourse.dve_uop import DveOpSpec as _DveOpSpec


def _register_dual_axpy():
    """Runtime-register a custom DVE op: out = in0*s0 + in1*s1 (TTSS).
    One DVE pass covers two bilinear corners (vs scale+stt = two ops)."""
    name = 'DUAL_AXPY_ANT'
    for op in _dops.OPS:
        if op.name == name:
            return op
    spec = _Spec(
        body=_Src0 * _C0 + _Src1 * _C1,
        reference=lambda in0, in1, s0, s1, imm2: (
            in0.astype(np.float32) * s0 + in1.astype(np.float32) * s1),
    )
    row = _dops._CUSTOM_DVE_ROW_BASE + len(_dops.OPS)
    _dops._SUB_OPCODE_FOR_NAME[name] = row
    shas = {}
    for ver in ('v3', 'v4'):
        uops = _dve_lower(spec, ver=ver)
        shas[ver] = _DveOpSpec(name=name, opcode=row, uops=uops,
                               rd1_en=True).sha(ver)
    op = _dops.DveOp(name, spec, subdim=False, uops_sha=shas,
                     perf_en={'v3': True, 'v4': True})
    _dops.OPS.append(op)
    _dops.CUSTOM_DVE_SPECS[name] = spec
    return op


_DUAL_AXPY = _register_dual_axpy()

F16 = mybir.dt.float16
F32 = mybir.dt.float32
I16 = mybir.dt.int16
I8 = mybir.dt.int8
AF = mybir.ActivationFunctionType
ALU = mybir.AluOpType

C = 256          # input channels
O = 256          # per-branch output channels
KK = 9           # 3x3 taps
NCLS = 324
G2 = 512         # cat channels
CKT = 18         # branch contraction tiles (9 taps x 2 c-halves)
CFT = 36         # 3x3 contraction tiles (9 taps x 4 ic-tiles)
XS = 32.0        # int8 quantization scale for x

# blend stream split per round (72 groups of (br,kk,gi)). GPSIMD (Pool) ALU
# ops measure ~1.7us each on HW (unusable); ACT cannot add tensors. The
# custom DUAL_AXPY op covers two corners per DVE pass:
#   axpt: 2 DVE dual-axpy halves, combined by accumulating PE transposes
#   axp:  2 DVE dual-axpy halves + DVE TT-add combine
#   dve3: 2 ACT scales + 2 DVE stt + DVE TT-add
#   dve2: DVE scale + 3 DVE stt           (fully DVE-resident)
N_AXPT = 0       # axpt retired: PE transpose ignores PSUM accumulation on HW
N_AXP = 72
N_3 = 0
BLOCK = 8        # groups per emission block (dependent ops 8 apart)
VBUFS = 4        # gather tile ring depth (f16 tiles, 8KB each)
EVAC_ACT = True  # sa evacuation engine: ACT (True) or DVE (False)


def _corner_geom(dm, Hd):
    """y0, x0 (int), corner weights [4,KK,H,W] for one deform map [18,H,W]."""
    Wd = Hd
    off = dm.reshape(KK, 2, Hd, Wd)
    dy, dx = off[:, 0], off[:, 1]
    ky = np.repeat(np.arange(3), 3).astype(np.float32)
    kx = np.tile(np.arange(3), 3).astype(np.float32)
    py = np.arange(Hd, dtype=np.float32)[None, :, None] + (ky - 1)[:, None, None] + dy
    px = np.arange(Wd, dtype=np.float32)[None, None, :] + (kx - 1)[:, None, None] + dx
    y0 = np.floor(py)
    x0 = np.floor(px)
    fy = (py - y0).astype(np.float32)
    fx = (px - x0).astype(np.float32)
    w00 = (1 - fy) * (1 - fx)
    w10 = fy * (1 - fx)
    w01 = (1 - fy) * fx
    w11 = fy * fx
    return (y0.astype(np.int64), x0.astype(np.int64),
            np.stack([w00, w10, w01, w11], 0))


# ---------------------------------------------------------------- host prep
def host_prep(x, dm0, dm1, w_dc0, w_dc1, w_cc, b_cc, w_f, b_f, Hd, P):
    """Per-core input prep. x: [C,Hd,Hd] fp32. P: global pad. Returns dict."""
    Wd = Hd
    NPOS = Hd * Wd
    NG = NPOS // 128
    NR = NG // 4

    geos = [_corner_geom(dm0, Hd), _corner_geom(dm1, Hd)]
    H2 = Hd + 2 * P
    W2 = Wd + 2 * P
    R = H2 * W2
    assert R <= 32766, f"pad too large: P={P}"

    # padded image, f16, HWC; one extra row so row pairs exist
    xp = np.zeros((H2 + 1, W2, C), np.float16)
    xp[P:P + Hd, P:P + Wd, :] = np.transpose(
        np.asarray(x, np.float32), (1, 2, 0)).astype(np.float16)
    x2 = np.concatenate([xp[:H2], xp[1:H2 + 1]], axis=2).reshape(R, 2 * C)

    # indices: clip fully-OOB cases into the zero border (contributions are 0)
    idx_cols = 2 * KK * NR * 32
    idx_sb = np.zeros((128, idx_cols), np.int16)
    wts = np.zeros((128, 2 * KK * 4 * NG), np.float32)
    for br in range(2):
        y0, x0, w4 = geos[br]
        y0c = np.clip(y0, -P, Hd - 1 + P)
        x0c = np.clip(x0, -P, Wd - 2 + P)
        ridx = ((y0c + P) * W2 + (x0c + P)).astype(np.int64)
        assert ridx.min() >= 0 and ridx.max() <= R - 2
        rflat = ridx.reshape(KK, NPOS)
        wflat = w4.reshape(4, KK, NPOS)
        for kk in range(KK):
            for r in range(NR):
                chunk = rflat[kk, r * 512:(r + 1) * 512].astype(np.int16)
                wrap = chunk.reshape(32, 16).T               # [16,32] col-major
                col0 = (br * KK + kk) * (NR * 32) + r * 32
                idx_sb[:, col0:col0 + 32] = np.tile(wrap, (8, 1))
            for cr in range(4):
                cols = wflat[cr, kk].reshape(NG, 128).T      # [128, NG]
                col0 = ((br * KK + kk) * 4 + cr) * NG
                wts[:, col0:col0 + NG] = cols

    # branch weights W': [2*18, 128, 256] fp16  (ck tile = kk*2 + chalf)
    wp = np.zeros((2, CKT, 128, O), np.float16)
    for br, wdc in enumerate((w_dc0, w_dc1)):
        w3 = wdc.reshape(O, C, KK)                           # [o, c, kk]
        for kk in range(KK):
            for ch in range(2):
                blk = w3[:, ch * 128:(ch + 1) * 128, kk]     # [o, 128]
                wp[br, kk * 2 + ch] = blk.T.astype(np.float16)

    # 1x1 weights: [4, 128, 512] fp16
    wcc = np.zeros((4, 128, G2), np.float16)
    for ic in range(4):
        wcc[ic] = w_cc[:, ic * 128:(ic + 1) * 128, 0, 0].T.astype(np.float16)

    # 3x3 weights: [36, 128, 324] fp16 (tile t = tap*4 + ic_tile)
    wf = np.zeros((CFT, 128, NCLS), np.float16)
    for tap in range(KK):
        for ic in range(4):
            blk = w_f[:, ic * 128:(ic + 1) * 128, tap // 3, tap % 3]
            wf[tap * 4 + ic] = blk.T.astype(np.float16)

    bcc = np.zeros((128, 4), np.float32)
    for ic in range(4):
        bcc[:, ic] = b_cc[ic * 128:(ic + 1) * 128]
    bf = np.zeros((128, 3), np.float32)
    bf_pad = np.zeros(384, np.float32)
    bf_pad[:NCLS] = b_f
    for ot in range(3):
        bf[:, ot] = bf_pad[ot * 128:(ot + 1) * 128]

    return {
        'x2': x2, 'idx': idx_sb, 'wts': wts, 'wp': wp.reshape(2 * CKT, 128, O),
        'wcc': wcc, 'wf': wf, 'bcc': bcc, 'bf': bf,
        'ident': np.eye(128, dtype=np.float16),
    }


def _spread(counts):
    """Evenly interleave class labels; counts: dict label -> count."""
    total = sum(counts.values())
    acc = dict.fromkeys(counts, 0.0)
    out = []
    for i in range(1, total + 1):
        k = max(counts, key=lambda k: counts[k] * i / total - acc[k])
        acc[k] += 1.0
        out.append(k)
    return out


# ------------------------------------------------------------- kernel build
def build_kernel(Hd, R, mode='full', reps=1, n_axpt=N_AXPT, n_axp=N_AXP,
                 n_3=N_3, evac_act=EVAC_ACT):
    """Build the Bacc kernel for image size Hd (R = padded x2 rows).
    mode: 'full' | 'gatheronly' | 'noblend' | 'notr2' | 'noconv3'.
    reps: repeat the whole pipeline (for marginal-cost timing)."""
    Wd = Hd
    NPOS = Hd * Wd
    NG = NPOS // 128
    NR = NG // 4          # rounds of 512 positions
    H3 = Hd + 2
    N3 = H3 * H3
    RT3 = min(H3, 512 // H3)          # padded rows per 3x3 n-tile
    NT3 = (H3 + RT3 - 1) // RT3
    RPR = 512 // Wd       # image rows per round

    nc = bacc.Bacc(None, target_bir_lowering=False, num_swdge_queues=4)

    x2_d = nc.dram_tensor('x2', [R, 2 * C], F16, kind='ExternalInput')
    idx_d = nc.dram_tensor('idx', [128, 2 * KK * NR * 32], I16, kind='ExternalInput')
    wts_d = nc.dram_tensor('wts', [128, 2 * KK * 4 * NG], F32, kind='ExternalInput')
    wp_d = nc.dram_tensor('wp', [2 * CKT, 128, O], F16, kind='ExternalInput')
    wcc_d = nc.dram_tensor('wcc', [4, 128, G2], F16, kind='ExternalInput')
    wf_d = nc.dram_tensor('wf', [CFT, 128, NCLS], F16, kind='ExternalInput')
    bcc_d = nc.dram_tensor('bcc', [128, 4], F32, kind='ExternalInput')
    bf_d = nc.dram_tensor('bf', [128, 3], F32, kind='ExternalInput')
    id_d = nc.dram_tensor('ident', [128, 128], F16, kind='ExternalInput')
    out_d = nc.dram_tensor('out', [NCLS, NPOS], F32, kind='ExternalOutput')
    dbg_d = None
    if mode == 'gatheronly':
        dbg_d = nc.dram_tensor('dbg', [128, NR * 2 * KK * 64], F16,
                               kind='ExternalOutput')

    # overlapping-window AP over x2: [R-1 rows, 1024 i8] stepping one row (512)
    win = x2_d[:, :].copy()
    win.ap = bass_rust.VecI64Pair([[2 * C, R - 1], [1, 4 * C]])

    # blend stream pattern over the 72 (br,kk,gi) groups of a round
    n_2 = 72 - n_axpt - n_axp - n_3
    pattern = _spread({'axpt': n_axpt, 'axp': n_axp, 'dve3': n_3, 'dve2': n_2})

    with TileContext(nc) as tc:
        with tc.tile_pool(name='const', bufs=1) as cpool, \
             tc.tile_pool(name='vg', bufs=VBUFS) as vpool, \
             tc.tile_pool(name='st', bufs=3) as stpool, \
             tc.tile_pool(name='sasm', bufs=1) as sapool, \
             tc.tile_pool(name='cat', bufs=2) as catpool, \
             tc.tile_pool(name='hbuf', bufs=1) as hpool, \
             tc.tile_pool(name='outs', bufs=2) as opool, \
             tc.tile_pool(name='ptr', bufs=2, space='PSUM') as trppool, \
             tc.tile_pool(name='pcat', bufs=2, space='PSUM') as catppool, \
             tc.tile_pool(name='ph', bufs=2, space='PSUM') as hppool, \
             tc.tile_pool(name='pf', bufs=2, space='PSUM') as fppool:

            # ---- constants ----
            idx_t = cpool.tile([128, 2 * KK * NR * 32], I16, tag='idx')
            nc.sync.dma_start(idx_t[:], idx_d[:])
            wts_t = cpool.tile([128, 2 * KK * 4 * NG], F32, tag='wts')
            nc.sync.dma_start(wts_t[:], wts_d[:])
            ident = cpool.tile([128, 128], F16, tag='ident')
            nc.sync.dma_start(ident[:], id_d[:])
            wp_t = []
            for i in range(2 * CKT):
                t = cpool.tile([128, O], F16, tag=f'wp{i}')
                nc.sync.dma_start(t[:], wp_d[i])
                wp_t.append(t)
            wcc_t = []
            for ic in range(4):
                t = cpool.tile([128, G2], F16, tag=f'wcc{ic}')
                nc.sync.dma_start(t[:], wcc_d[ic])
                wcc_t.append(t)
            bcc_t = cpool.tile([128, 4], F32, tag='bcc')
            nc.sync.dma_start(bcc_t[:], bcc_d[:])
            bf_t = cpool.tile([128, 3], F32, tag='bf')
            nc.sync.dma_start(bf_t[:], bf_d[:])
            wf_t = []
            for i in range(CFT):
                t = cpool.tile([128, NCLS], F16, tag=f'wf{i}')
                nc.sync.dma_start(t[:], wf_d[i])
                wf_t.append(t)

            # ---- padded h grid (zeroed; guard margins for 3x3 shifts) ----
            h_t = []
            for ic in range(4):
                t = hpool.tile([128, N3 + 136], F16, tag=f'h{ic}')
                nc.vector.memset(t[:], 0.0)
                h_t.append(t)

            def wcol(br, kk, cr, g):
                return ((br * KK + kk) * 4 + cr) * NG + g

            OT3 = [(0, 128), (128, 128), (256, 68)]

            def emit_3x3_o(nt, o, rep):
                r0 = nt * RT3
                nrows = min(RT3, H3 - r0)
                nsz = nrows * H3
                n0 = r0 * H3
                obase, orows = OT3[o]
                pf = fppool.tile([128, 512], F32, tag='pf',
                                 name=f'pf{rep}_{nt}_{o}')
                for j in range(CFT):
                    tap, ic = j // 4, j % 4
                    ky, kx = tap // 3, tap % 3
                    off = (ky - 1) * H3 + (kx - 1)
                    nc.tensor.matmul(
                        pf[:orows, :nsz],
                        wf_t[j][:, obase:obase + orows],
                        h_t[ic][:, 68 + off + n0: 68 + off + n0 + nsz],
                        start=(j == 0), stop=(j == CFT - 1))
                stg = opool.tile([128, 512], F32, tag='stg',
                                 name=f'stg{rep}_{nt}_{o}')
                nc.scalar.activation(stg[:orows, :nsz], pf[:orows, :nsz],
                                     AF.Identity, bias=bf_t[:orows, o:o + 1])
                vr0 = max(1, r0)
                vr1 = min(H3 - 2, r0 + nrows - 1)
                nvr = vr1 - vr0 + 1
                if nvr <= 0:
                    return
                src2 = stg[:, :].copy()
                pstep = src2.ap[0][0]
                src2.offset = src2.offset + (vr0 - r0) * H3 + 1
                src2.ap = bass_rust.VecI64Pair(
                    [[pstep, orows], [H3, nvr], [1, Wd]])
                nc.sync.dma_start(
                    out_d[obase:obase + orows,
                          (vr0 - 1) * Wd:(vr0 - 1 + nvr) * Wd], src2)

            # ---- per-round emission helpers ----
            def emit_blend_block(block, vtiles, sasm, r, pending_evacs):
                """block: list of (slot, (br,kk,gi), kind). Waves of
                independent ops (dependent ops sit len(block) apart per
                engine queue), then PE transposes into one [128,1024] f16
                PSUM tile per (br,kk). pending_evacs: sa-evac closures from
                the previous block."""
                # slot-packed tiles: one [128, 4C] pair per (br,kk) in the
                # block, so the half-combine can be a single wide TT
                bks = []
                for _, (br, kk, _), _ in block:
                    if (br, kk) not in bks:
                        bks.append((br, kk))
                packs = {}
                for bslot, bk in enumerate(bks):
                    stc4 = stpool.tile([128, 4 * C], F16, tag=f'st{bslot}',
                                       name=f'st{bslot}_{r}_{bk[0]}_{bk[1]}')
                    t24 = stpool.tile([128, 4 * C], F16, tag=f'tm{bslot}',
                                      name=f'tm{bslot}_{r}_{bk[0]}_{bk[1]}')
                    packs[bk] = (stc4, t24)
                tiles = {}
                for slot, (br, kk, gi), kind in block:
                    stc4, t24 = packs[(br, kk)]
                    tiles[slot] = (stc4[:, gi * C:(gi + 1) * C],
                                   t24[:, gi * C:(gi + 1) * C])

                def w(br, kk, cr, gi):
                    g = r * 4 + gi
                    cl = wcol(br, kk, cr, g)
                    return wts_t[:, cl:cl + 1]

                # wave 1: corners 0+1 -> stc
                for slot, (br, kk, gi), kind in block:
                    stc, _ = tiles[slot]
                    v = vtiles[(br, kk)]
                    if kind in ('axpt', 'axp'):
                        nc.vector._custom_dve(
                            _DUAL_AXPY, out=stc[:], in0=v[:, gi, 0:C],
                            in1=v[:, gi, C:2 * C],
                            s0=w(br, kk, 0, gi), s1=w(br, kk, 1, gi))
                    elif kind == 'dve3':
                        nc.scalar.activation(stc[:], v[:, gi, 0:C], AF.Copy,
                                             scale=w(br, kk, 0, gi))
                    else:
                        nc.vector.tensor_scalar_mul(stc[:], v[:, gi, 0:C],
                                                    w(br, kk, 0, gi))
                # wave 2: corners 2(+3) -> t2
                for slot, (br, kk, gi), kind in block:
                    stc, t2 = tiles[slot]
                    v = vtiles[(br, kk)]
                    if kind in ('axpt', 'axp'):
                        nc.vector._custom_dve(
                            _DUAL_AXPY, out=t2[:], in0=v[:, gi, 2 * C:3 * C],
                            in1=v[:, gi, 3 * C:4 * C],
                            s0=w(br, kk, 2, gi), s1=w(br, kk, 3, gi))
                    elif kind == 'dve3':
                        nc.scalar.activation(t2[:], v[:, gi, 2 * C:3 * C],
                                             AF.Copy, scale=w(br, kk, 2, gi))
                    else:
                        nc.vector.scalar_tensor_tensor(
                            stc[:], v[:, gi, C:2 * C], w(br, kk, 1, gi),
                            stc[:], ALU.mult, ALU.add)
                # previous block's sa evacs: PE transposes are long done
                for ev in pending_evacs:
                    ev()
                pending_evacs.clear()
                # wave 3: dve3: corner1 -> stc; dve2: corner2 -> stc
                for slot, (br, kk, gi), kind in block:
                    stc, t2 = tiles[slot]
                    v = vtiles[(br, kk)]
                    if kind == 'dve3':
                        nc.vector.scalar_tensor_tensor(
                            stc[:], v[:, gi, C:2 * C], w(br, kk, 1, gi),
                            stc[:], ALU.mult, ALU.add)
                    elif kind == 'dve2':
                        nc.vector.scalar_tensor_tensor(
                            stc[:], v[:, gi, 2 * C:3 * C], w(br, kk, 2, gi),
                            stc[:], ALU.mult, ALU.add)
                # wave 4: dve3: corner3 -> t2; dve2: corner3 -> stc
                for slot, (br, kk, gi), kind in block:
                    stc, t2 = tiles[slot]
                    v = vtiles[(br, kk)]
                    if kind == 'dve3':
                        nc.vector.scalar_tensor_tensor(
                            t2[:], v[:, gi, 3 * C:4 * C], w(br, kk, 3, gi),
                            t2[:], ALU.mult, ALU.add)
                    elif kind == 'dve2':
                        nc.vector.scalar_tensor_tensor(
                            stc[:], v[:, gi, 3 * C:4 * C], w(br, kk, 3, gi),
                            stc[:], ALU.mult, ALU.add)
                # wave 5: combine halves on DVE — one wide TT per all-axp
                # (br,kk), per-group TT/stt otherwise
                kind_of = {}
                tiles_of = {}
                for slot, g, kind in block:
                    kind_of[g] = kind
                    tiles_of[g] = tiles[slot]
                for br, kk in bks:
                    kinds = [kind_of[(br, kk, gi)] for gi in range(4)]
                    stc4, t24 = packs[(br, kk)]
                    if all(k in ('axp', 'axpt') for k in kinds):
                        nc.vector.tensor_tensor(stc4[:], stc4[:], t24[:],
                                                ALU.add)
                    else:
                        for gi in range(4):
                            if kinds[gi] in ('axp', 'axpt', 'dve3'):
                                stc, t2 = tiles_of[(br, kk, gi)]
                                nc.vector.tensor_tensor(stc[:], stc[:], t2[:],
                                                        ALU.add)
                # transposes: per completed (br,kk): gi/ch-blocks through PE
                # (identity matmul) into one [128,1024] f16 PSUM tile
                if mode == 'notr2':
                    return
                for br, kk in bks:
                    stc4, _ = packs[(br, kk)]
                    ptr = trppool.tile([128, 1024], F16, tag='ptr',
                                       name=f'ptr{r}_{br}_{kk}')
                    for ch in range(2):
                        for gi in range(4):
                            nc.tensor.transpose(
                                ptr[:, ch * 512 + gi * 128:
                                    ch * 512 + (gi + 1) * 128],
                                stc4[:, gi * C + ch * 128:
                                     gi * C + (ch + 1) * 128],
                                ident[:])
                    sa = sasm[(br, kk)]
                    if evac_act:
                        pending_evacs.append(
                            lambda sa=sa, ptr=ptr: nc.scalar.activation(
                                sa[:], ptr[:], AF.Copy))
                    else:
                        pending_evacs.append(
                            lambda sa=sa, ptr=ptr: nc.vector.tensor_copy(
                                sa[:], ptr[:]))

            def emit_einsum_br(br, sasm, pc_tiles, r):
                for o in range(2):
                    pc = catppool.tile([128, 512], F32, tag='pcat',
                                       name=f'pc{r}_{br}_{o}')
                    pc_tiles[(br, o)] = pc
                    for ck in range(CKT):
                        kk, ch = ck // 2, ck % 2
                        nc.tensor.matmul(
                            pc[:],
                            wp_t[br * CKT + ck][:, o * 128:(o + 1) * 128],
                            sasm[(br, kk)][:, ch * 512:(ch + 1) * 512],
                            start=(ck == 0), stop=(ck == CKT - 1))

            def emit_evac_br(br, pc_tiles, cat_tiles):
                for o in range(2):
                    ic = br * 2 + o
                    nc.scalar.activation(cat_tiles[ic][:], pc_tiles[(br, o)][:],
                                         AF.Copy)

            def make_tail(r, rep_r, cat_tiles, pc_tiles):
                """Round-r tail: head closure (evac br1 + 1x1 + h stores)
                plus a list of fine-grained 3x3 PE chunks, one per (nt, o),
                to interleave between the next round's blend blocks."""
                rep = rep_r // NR

                def head():
                    emit_evac_br(1, pc_tiles, cat_tiles)
                    for o in range(4):
                        ph = hppool.tile([128, 512], F32, tag='ph',
                                         name=f'ph{rep_r}_{o}')
                        for ic in range(4):
                            nc.tensor.matmul(
                                ph[:], wcc_t[ic][:, o * 128:(o + 1) * 128],
                                cat_tiles[ic][:], start=(ic == 0), stop=(ic == 3))
                        dst = h_t[o][:, :].copy()
                        pstep = dst.ap[0][0]
                        dst.offset = dst.offset + 68 + (r * RPR + 1) * H3 + 1
                        dst.ap = bass_rust.VecI64Pair(
                            [[pstep, 128], [H3, RPR], [1, Wd]])
                        nc.scalar.activation(dst, ph[:], AF.Identity,
                                             bias=bcc_t[:, o:o + 1])

                chunks = []
                if mode != 'noconv3':
                    ready = (r + 1) * RPR
                    for nt in range(NT3):
                        last = min(nt * RT3 + min(RT3, H3 - nt * RT3), H3 - 2)
                        prev_ready = r * RPR if r > 0 else -1
                        if last <= ready and not (last <= prev_ready):
                            for o in range(3):
                                chunks.append(
                                    lambda nt=nt, o=o: emit_3x3_o(nt, o, rep))
                return head, chunks

            # ---- main loop over rounds of 512 positions ----
            gidx = 0          # global gather counter: queue = gidx % 4 stays
            # aligned with Tile's DMASW lane round-robin (lane = gidx % 8),
            # so each sem lane only ever sees one SWDGE queue.
            pending_tail = None
            pending_pe = []
            for rep_r in range(reps * NR):
                r = rep_r % NR
                vtiles = {}
                for br in range(2):
                    for kk in range(KK):
                        col0 = (br * KK + kk) * (NR * 32) + r * 32
                        v = vpool.tile([128, 4, 4 * C], F16, tag='v',
                                       name=f'v{rep_r}_{br}_{kk}')
                        nc.gpsimd.dma_gather(
                            v[:], win, idx_t[:, col0:col0 + 32],
                            512, 512, 4 * C, elem_step=2 * C,
                            queue_num=gidx % 4)
                        gidx += 1
                        vtiles[(br, kk)] = v
                if mode == 'gatheronly':
                    for br in range(2):
                        for kk in range(KK):
                            col = ((r * 2 + br) * KK + kk) * 64
                            nc.sync.dma_start(
                                dbg_d[:, col:col + 64],
                                vtiles[(br, kk)][:, 0, 0:64])
                    continue
                if mode == 'noblend':
                    continue

                # group plan for this round
                groups = [(br, kk, gi)
                          for br in range(2) for kk in range(KK)
                          for gi in range(4)]
                sasm = {}
                if mode not in ('notr2',):
                    for br in range(2):
                        for kk in range(KK):
                            sasm[(br, kk)] = sapool.tile(
                                [128, 1024], F16, tag=f'sa{br}_{kk}',
                                name=f'sa{br}_{kk}_{rep_r}')

                cat_tiles = {}
                pc_tiles = {}
                for ic in range(4):
                    cat_tiles[ic] = catpool.tile([128, 512], F16, tag=f'cat{ic}',
                                                 name=f'cat{ic}_{rep_r}')

                # per-branch: blend blocks then einsum; pending tail from the
                # previous round flushes after the 2nd block of br0, and the
                # br0 cat evac lands after the 2nd block of br1.
                blk_count = 0
                pending_evacs = []
                for br in range(2):
                    gset = [g for g in groups if g[0] == br]
                    blocks = [gset[i:i + BLOCK] for i in range(0, len(gset), BLOCK)]
                    for bi, blk in enumerate(blocks):
                        block = []
                        for slot, g in enumerate(blk):
                            gidx72 = (g[0] * KK + g[1]) * 4 + g[2]
                            block.append((slot, g, pattern[gidx72]))
                        emit_blend_block(block, vtiles, sasm, r, pending_evacs)
                        blk_count += 1
                        if blk_count == 2 and pending_tail is not None:
                            pending_tail()
                            pending_tail = None
                        elif blk_count >= 3 and pending_pe:
                            pending_pe.pop(0)()
                        if br == 1 and bi == 1 and mode not in ('notr2',):
                            emit_evac_br(0, pc_tiles, cat_tiles)
                    if mode in ('notr2',):
                        continue
                    for ev in pending_evacs:
                        ev()
                    pending_evacs.clear()
                    emit_einsum_br(br, sasm, pc_tiles, r)

                if mode in ('notr2',):
                    continue
                pending_tail, new_chunks = make_tail(r, rep_r, cat_tiles,
                                                     pc_tiles)
                pending_pe.extend(new_chunks)

            if pending_tail is not None:
                pending_tail()
                pending_tail = None
            for chunk in pending_pe:
                chunk()
            pending_pe = []

    nc.compile()
    return nc


# ----------------------------------------------------------------- driver
_CACHE = {}


def _get_kernel(Hd, R):
    key = (Hd, R)
    if key not in _CACHE:
        _CACHE[key] = build_kernel(Hd, R)
    return _CACHE[key]


def global_pad(deform_map0, deform_map1, Hd):
    """Common pad P across the whole batch (all cores share one NEFF)."""
    P = 2
    for dms in (deform_map0, deform_map1):
        for b in range(dms.shape[0]):
            y0, x0, _ = _corner_geom(np.asarray(dms[b], np.float32), Hd)
            P = max(P, int(-y0.min()), int(y0.max() - 62),
                    int(-x0.min()), int(x0.max() - 62))
    return P


def prep_all(x, deform_map0, deform_map1, w_dc0, w_dc1, w_cc, b_cc, w_f, b_f):
    x = np.asarray(x, np.float32)
    Hd = x.shape[2]
    P = global_pad(np.asarray(deform_map0, np.float32),
                   np.asarray(deform_map1, np.float32), Hd)
    in_maps = []
    for b in range(x.shape[0]):
        m = host_prep(x[b], np.asarray(deform_map0[b], np.float32),
                      np.asarray(deform_map1[b], np.float32),
                      np.asarray(w_dc0, np.float32), np.asarray(w_dc1, np.float32),
                      np.asarray(w_cc, np.float32), np.asarray(b_cc, np.float32),
                      np.asarray(w_f, np.float32), np.asarray(b_f, np.float32),
                      Hd, P)
        in_maps.append(m)
    R = in_maps[0]['x2'].shape[0]
    return in_maps, Hd, R


def kernel(x, deform_map0, deform_map1, w_dc0, w_dc1, w_cc, b_cc, w_f, b_f):
    from concourse.bass_utils import run_bass_kernel_spmd
    in_maps, Hd, R = prep_all(x, deform_map0, deform_map1, w_dc0, w_dc1,
                              w_cc, b_cc, w_f, b_f)
    B = len(in_maps)
    nc = _get_kernel(Hd, R)
    res = run_bass_kernel_spmd(nc, in_maps, core_ids=list(range(B)))
    out = np.stack([res.results[b]['out'].reshape(NCLS, Hd, Hd) for b in range(B)])
    return out.astype(np.float32)


# revision 52
# speedup vs baseline: 3.3491x; 1.0137x over previous
"""Trainium2 Bass kernel for nn_DeformableInception.

Architecture (per core, one batch element; batch-parallel over 8 cores):
  1. Host prep: gather indices + bilinear corner weights from deform maps;
     f16 padded vertical-pair image so one 2KB gather descriptor fetches
     all 4 bilinear corners x 256 channels.
  2. dma_gather (SWDGE, 4 queues round-robin): positions-on-partitions
     corner blocks, f16.
  3. Bilinear blend on DVE via a runtime-registered custom DUAL_AXPY op
     (out = v0*w0 + v1*w1, two corners per pass) + one wide TT-add per
     (br,kk); emitted in 8-group waves so dependent ops sit 8 apart in
     the DVE queue (no back-to-back semaphore stalls).
  4. Transpose S^T -> S via PE identity matmuls into one [128,1024] f16
     PSUM tile per (br,kk), with one wide ACT evacuation each, emitted
     per blend block to keep the PE stream dense and overlapped.
  5. Branch einsum per br half-round: W' [ck,o-block] stationary x S
     [ck,pos512] -> cat PSUM fp32; evac + 1x1 + h-store + 3x3 run as a
     software-pipelined tail one round behind (keeps ACT from stalling on
     PE results).
  6. 3x3 conv via shifted free-dim APs over a zero-padded h grid, output
     rows DMA'd out as they complete.
All matmuls fp16 operands with fp32 PSUM accumulation.
"""
import sys
import numpy as np

sys.path.insert(0, '/opt/trn_rl_repo')

import bass_rust
import concourse.bacc as bacc
import concourse.bass as bass
import concourse.mybir as mybir
from concourse.tile import TileContext
from concourse import dve_ops as _dops
from concourse.dve_spec import (
    Spec as _Spec, Src0 as _Src0, Src1 as _Src1, C0 as _C0, C1 as _C1,
    lower as _dve_lower,
)
from concourse.dve_uop import DveOpSpec as _DveOpSpec


def _register_dual_axpy():
    """Runtime-register a custom DVE op: out = in0*s0 + in1*s1 (TTSS).
    One DVE pass covers two bilinear corners (vs scale+stt = two ops)."""
    name = 'DUAL_AXPY_ANT'
    for op in _dops.OPS:
        if op.name == name:
            return op
    spec = _Spec(
        body=_Src0 * _C0 + _Src1 * _C1,
        reference=lambda in0, in1, s0, s1, imm2: (
            in0.astype(np.float32) * s0 + in1.astype(np.float32) * s1),
    )
    row = _dops._CUSTOM_DVE_ROW_BASE + len(_dops.OPS)
    _dops._SUB_OPCODE_FOR_NAME[name] = row
    shas = {}
    for ver in ('v3', 'v4'):
        uops = _dve_lower(spec, ver=ver)
        shas[ver] = _DveOpSpec(name=name, opcode=row, uops=uops,
                               rd1_en=True).sha(ver)
    op = _dops.DveOp(name, spec, subdim=False, uops_sha=shas,
                     perf_en={'v3': True, 'v4': True})
    _dops.OPS.append(op)
    _dops.CUSTOM_DVE_SPECS[name] = spec
    return op


_DUAL_AXPY = _register_dual_axpy()

F16 = mybir.dt.float16
F32 = mybir.dt.float32
I16 = mybir.dt.int16
I8 = mybir.dt.int8
AF = mybir.ActivationFunctionType
ALU = mybir.AluOpType

C = 256          # input channels
O = 256          # per-branch output channels
KK = 9           # 3x3 taps
NCLS = 324
G2 = 512         # cat channels
CKT = 18         # branch contraction tiles (9 taps x 2 c-halves)
CFT = 36         # 3x3 contraction tiles (9 taps x 4 ic-tiles)

# blend stream split per round (72 groups of (br,kk,gi)). GPSIMD (Pool) ALU
# ops measure ~1.7us each on HW (unusable); ACT cannot add tensors. The
# custom DUAL_AXPY op covers two corners per DVE pass:
#   axpt: 2 DVE dual-axpy halves, combined by accumulating PE transposes
#   axp:  2 DVE dual-axpy halves + DVE TT-add combine
#   dve3: 2 ACT scales + 2 DVE stt + DVE TT-add
#   dve2: DVE scale + 3 DVE stt           (fully DVE-resident)
N_AXPT = 0       # axpt retired: PE transpose ignores PSUM accumulation on HW
N_AXP = 72
N_3 = 0
BLOCK = 8        # groups per emission block (dependent ops 8 apart)
VBUFS = 4        # gather tile ring depth (f16 tiles, 8KB each)
EVAC_ACT = True  # sa evacuation engine: ACT (True) or DVE (False)


def _corner_geom(dm, Hd):
    """y0, x0 (int), corner weights [4,KK,H,W] for one deform map [18,H,W]."""
    Wd = Hd
    off = dm.reshape(KK, 2, Hd, Wd)
    dy, dx = off[:, 0], off[:, 1]
    ky = np.repeat(np.arange(3), 3).astype(np.float32)
    kx = np.tile(np.arange(3), 3).astype(np.float32)
    py = np.arange(Hd, dtype=np.float32)[None, :, None] + (ky - 1)[:, None, None] + dy
    px = np.arange(Wd, dtype=np.float32)[None, None, :] + (kx - 1)[:, None, None] + dx
    y0 = np.floor(py)
    x0 = np.floor(px)
    fy = (py - y0).astype(np.float32)
    fx = (px - x0).astype(np.float32)
    w00 = (1 - fy) * (1 - fx)
    w10 = fy * (1 - fx)
    w01 = (1 - fy) * fx
    w11 = fy * fx
    return (y0.astype(np.int64), x0.astype(np.int64),
            np.stack([w00, w10, w01, w11], 0))


# ---------------------------------------------------------------- host prep
def host_prep(x, dm0, dm1, w_dc0, w_dc1, w_cc, b_cc, w_f, b_f, Hd, P):
    """Per-core input prep. x: [C,Hd,Hd] fp32. P: global pad. Returns dict."""
    Wd = Hd
    NPOS = Hd * Wd
    NG = NPOS // 128
    NR = NG // 4

    geos = [_corner_geom(dm0, Hd), _corner_geom(dm1, Hd)]
    H2 = Hd + 2 * P
    W2 = Wd + 2 * P
    R = H2 * W2
    assert R <= 32766, f"pad too large: P={P}"

    # padded image, f16, HWC; one extra row so row pairs exist
    xp = np.zeros((H2 + 1, W2, C), np.float16)
    xp[P:P + Hd, P:P + Wd, :] = np.transpose(
        np.asarray(x, np.float32), (1, 2, 0)).astype(np.float16)
    x2 = np.concatenate([xp[:H2], xp[1:H2 + 1]], axis=2).reshape(R, 2 * C)

    # indices: clip fully-OOB cases into the zero border (contributions are 0)
    idx_cols = 2 * KK * NR * 32
    idx_sb = np.zeros((128, idx_cols), np.int16)
    wts = np.zeros((128, 2 * KK * 4 * NG), np.float32)
    for br in range(2):
        y0, x0, w4 = geos[br]
        y0c = np.clip(y0, -P, Hd - 1 + P)
        x0c = np.clip(x0, -P, Wd - 2 + P)
        ridx = ((y0c + P) * W2 + (x0c + P)).astype(np.int64)
        assert ridx.min() >= 0 and ridx.max() <= R - 2
        rflat = ridx.reshape(KK, NPOS)
        wflat = w4.reshape(4, KK, NPOS)
        for kk in range(KK):
            for r in range(NR):
                chunk = rflat[kk, r * 512:(r + 1) * 512].astype(np.int16)
                wrap = chunk.reshape(32, 16).T               # [16,32] col-major
                col0 = (br * KK + kk) * (NR * 32) + r * 32
                idx_sb[:, col0:col0 + 32] = np.tile(wrap, (8, 1))
            for cr in range(4):
                cols = wflat[cr, kk].reshape(NG, 128).T      # [128, NG]
                col0 = ((br * KK + kk) * 4 + cr) * NG
                wts[:, col0:col0 + NG] = cols

    # branch weights W': [2*18, 128, 256] fp16  (ck tile = kk*2 + chalf)
    wp = np.zeros((2, CKT, 128, O), np.float16)
    for br, wdc in enumerate((w_dc0, w_dc1)):
        w3 = wdc.reshape(O, C, KK)                           # [o, c, kk]
        for kk in range(KK):
            for ch in range(2):
                blk = w3[:, ch * 128:(ch + 1) * 128, kk]     # [o, 128]
                wp[br, kk * 2 + ch] = blk.T.astype(np.float16)

    # 1x1 weights: [4, 128, 512] fp16
    wcc = np.zeros((4, 128, G2), np.float16)
    for ic in range(4):
        wcc[ic] = w_cc[:, ic * 128:(ic + 1) * 128, 0, 0].T.astype(np.float16)

    # 3x3 weights: [36, 128, 324] fp16 (tile t = tap*4 + ic_tile)
    wf = np.zeros((CFT, 128, NCLS), np.float16)
    for tap in range(KK):
        for ic in range(4):
            blk = w_f[:, ic * 128:(ic + 1) * 128, tap // 3, tap % 3]
            wf[tap * 4 + ic] = blk.T.astype(np.float16)

    bcc = np.zeros((128, 4), np.float32)
    for ic in range(4):
        bcc[:, ic] = b_cc[ic * 128:(ic + 1) * 128]
    bf = np.zeros((128, 3), np.float32)
    bf_pad = np.zeros(384, np.float32)
    bf_pad[:NCLS] = b_f
    for ot in range(3):
        bf[:, ot] = bf_pad[ot * 128:(ot + 1) * 128]

    return {
        'x2': x2, 'idx': idx_sb, 'wts': wts, 'wp': wp.reshape(2 * CKT, 128, O),
        'wcc': wcc, 'wf': wf, 'bcc': bcc, 'bf': bf,
        'ident': np.eye(128, dtype=np.float16),
    }


def _spread(counts):
    """Evenly interleave class labels; counts: dict label -> count."""
    total = sum(counts.values())
    acc = dict.fromkeys(counts, 0.0)
    out = []
    for i in range(1, total + 1):
        k = max(counts, key=lambda k: counts[k] * i / total - acc[k])
        acc[k] += 1.0
        out.append(k)
    return out


# ------------------------------------------------------------- kernel build
def build_kernel(Hd, R, mode='full', reps=1, n_axpt=N_AXPT, n_axp=N_AXP,
                 n_3=N_3, evac_act=EVAC_ACT):
    """Build the Bacc kernel for image size Hd (R = padded x2 rows).
    mode: 'full' | 'gatheronly' | 'noblend' | 'notr2' | 'noconv3'.
    reps: repeat the whole pipeline (for marginal-cost timing)."""
    Wd = Hd
    NPOS = Hd * Wd
    NG = NPOS // 128
    NR = NG // 4          # rounds of 512 positions
    H3 = Hd + 2
    N3 = H3 * H3
    RT3 = min(H3, 512 // H3)          # padded rows per 3x3 n-tile
    NT3 = (H3 + RT3 - 1) // RT3
    RPR = 512 // Wd       # image rows per round

    nc = bacc.Bacc(None, target_bir_lowering=False, num_swdge_queues=4)

    x2_d = nc.dram_tensor('x2', [R, 2 * C], F16, kind='ExternalInput')
    idx_d = nc.dram_tensor('idx', [128, 2 * KK * NR * 32], I16, kind='ExternalInput')
    wts_d = nc.dram_tensor('wts', [128, 2 * KK * 4 * NG], F32, kind='ExternalInput')
    wp_d = nc.dram_tensor('wp', [2 * CKT, 128, O], F16, kind='ExternalInput')
    wcc_d = nc.dram_tensor('wcc', [4, 128, G2], F16, kind='ExternalInput')
    wf_d = nc.dram_tensor('wf', [CFT, 128, NCLS], F16, kind='ExternalInput')
    bcc_d = nc.dram_tensor('bcc', [128, 4], F32, kind='ExternalInput')
    bf_d = nc.dram_tensor('bf', [128, 3], F32, kind='ExternalInput')
    id_d = nc.dram_tensor('ident', [128, 128], F16, kind='ExternalInput')
    out_d = nc.dram_tensor('out', [NCLS, NPOS], F32, kind='ExternalOutput')
    dbg_d = None
    if mode == 'gatheronly':
        dbg_d = nc.dram_tensor('dbg', [128, NR * 2 * KK * 64], F16,
                               kind='ExternalOutput')

    # overlapping-window AP over x2: [R-1 rows, 1024 i8] stepping one row (512)
    win = x2_d[:, :].copy()
    win.ap = bass_rust.VecI64Pair([[2 * C, R - 1], [1, 4 * C]])

    # blend stream pattern over the 72 (br,kk,gi) groups of a round
    n_2 = 72 - n_axpt - n_axp - n_3
    pattern = _spread({'axpt': n_axpt, 'axp': n_axp, 'dve3': n_3, 'dve2': n_2})

    with TileContext(nc) as tc:
        with tc.tile_pool(name='const', bufs=1) as cpool, \
             tc.tile_pool(name='vg', bufs=VBUFS) as vpool, \
             tc.tile_pool(name='st', bufs=3) as stpool, \
             tc.tile_pool(name='sasm', bufs=1) as sapool, \
             tc.tile_pool(name='cat', bufs=2) as catpool, \
             tc.tile_pool(name='hbuf', bufs=1) as hpool, \
             tc.tile_pool(name='outs', bufs=2) as opool, \
             tc.tile_pool(name='ptr', bufs=2, space='PSUM') as trppool, \
             tc.tile_pool(name='pcat', bufs=2, space='PSUM') as catppool, \
             tc.tile_pool(name='ph', bufs=2, space='PSUM') as hppool, \
             tc.tile_pool(name='pf', bufs=2, space='PSUM') as fppool:

            # ---- constants ----
            idx_t = cpool.tile([128, 2 * KK * NR * 32], I16, tag='idx')
            nc.sync.dma_start(idx_t[:], idx_d[:])
            wts_t = cpool.tile([128, 2 * KK * 4 * NG], F32, tag='wts')
            nc.sync.dma_start(wts_t[:], wts_d[:])
            ident = cpool.tile([128, 128], F16, tag='ident')
            nc.sync.dma_start(ident[:], id_d[:])
            wp_t = []
            for i in range(2 * CKT):
                t = cpool.tile([128, O], F16, tag=f'wp{i}')
                nc.sync.dma_start(t[:], wp_d[i])
                wp_t.append(t)
            wcc_t = []
            for ic in range(4):
                t = cpool.tile([128, G2], F16, tag=f'wcc{ic}')
                nc.sync.dma_start(t[:], wcc_d[ic])
                wcc_t.append(t)
            bcc_t = cpool.tile([128, 4], F32, tag='bcc')
            nc.sync.dma_start(bcc_t[:], bcc_d[:])
            bf_t = cpool.tile([128, 3], F32, tag='bf')
            nc.sync.dma_start(bf_t[:], bf_d[:])
            wf_t = []
            for i in range(CFT):
                t = cpool.tile([128, NCLS], F16, tag=f'wf{i}')
                nc.sync.dma_start(t[:], wf_d[i])
                wf_t.append(t)

            # ---- padded h grid (zeroed; guard margins for 3x3 shifts) ----
            h_t = []
            for ic in range(4):
                t = hpool.tile([128, N3 + 136], F16, tag=f'h{ic}')
                nc.vector.memset(t[:], 0.0)
                h_t.append(t)

            def wcol(br, kk, cr, g):
                return ((br * KK + kk) * 4 + cr) * NG + g

            OT3 = [(0, 128), (128, 128), (256, 68)]

            def emit_3x3_o(nt, o, rep):
                r0 = nt * RT3
                nrows = min(RT3, H3 - r0)
                nsz = nrows * H3
                n0 = r0 * H3
                obase, orows = OT3[o]
                pf = fppool.tile([128, 512], F32, tag='pf',
                                 name=f'pf{rep}_{nt}_{o}')
                for j in range(CFT):
                    tap, ic = j // 4, j % 4
                    ky, kx = tap // 3, tap % 3
                    off = (ky - 1) * H3 + (kx - 1)
                    nc.tensor.matmul(
                        pf[:orows, :nsz],
                        wf_t[j][:, obase:obase + orows],
                        h_t[ic][:, 68 + off + n0: 68 + off + n0 + nsz],
                        start=(j == 0), stop=(j == CFT - 1))
                stg = opool.tile([128, 512], F32, tag='stg',
                                 name=f'stg{rep}_{nt}_{o}')
                nc.scalar.activation(stg[:orows, :nsz], pf[:orows, :nsz],
                                     AF.Identity, bias=bf_t[:orows, o:o + 1])
                vr0 = max(1, r0)
                vr1 = min(H3 - 2, r0 + nrows - 1)
                nvr = vr1 - vr0 + 1
                if nvr <= 0:
                    return
                src2 = stg[:, :].copy()
                pstep = src2.ap[0][0]
                src2.offset = src2.offset + (vr0 - r0) * H3 + 1
                src2.ap = bass_rust.VecI64Pair(
                    [[pstep, orows], [H3, nvr], [1, Wd]])
                nc.sync.dma_start(
                    out_d[obase:obase + orows,
                          (vr0 - 1) * Wd:(vr0 - 1 + nvr) * Wd], src2)

            # ---- per-round emission helpers ----
            def emit_blend_block(block, vtiles, sasm, r, pending_evacs):
                """block: list of (slot, (br,kk,gi), kind). Waves of
                independent ops (dependent ops sit len(block) apart per
                engine queue), then PE transposes into one [128,1024] f16
                PSUM tile per (br,kk). pending_evacs: sa-evac closures from
                the previous block."""
                # slot-packed tiles: one [128, 4C] pair per (br,kk) in the
                # block, so the half-combine can be a single wide TT
                bks = []
                for _, (br, kk, _), _ in block:
                    if (br, kk) not in bks:
                        bks.append((br, kk))
                packs = {}
                for bslot, bk in enumerate(bks):
                    stc4 = stpool.tile([128, 4 * C], F16, tag=f'st{bslot}',
                                       name=f'st{bslot}_{r}_{bk[0]}_{bk[1]}')
                    t24 = stpool.tile([128, 4 * C], F16, tag=f'tm{bslot}',
                                      name=f'tm{bslot}_{r}_{bk[0]}_{bk[1]}')
                    packs[bk] = (stc4, t24)
                tiles = {}
                for slot, (br, kk, gi), kind in block:
                    stc4, t24 = packs[(br, kk)]
                    tiles[slot] = (stc4[:, gi * C:(gi + 1) * C],
                                   t24[:, gi * C:(gi + 1) * C])

                def w(br, kk, cr, gi):
                    g = r * 4 + gi
                    cl = wcol(br, kk, cr, g)
                    return wts_t[:, cl:cl + 1]

                # wave 1: corners 0+1 -> stc
                for slot, (br, kk, gi), kind in block:
                    stc, _ = tiles[slot]
                    v = vtiles[(br, kk)]
                    if kind in ('axpt', 'axp'):
                        nc.vector._custom_dve(
                            _DUAL_AXPY, out=stc[:], in0=v[:, gi, 0:C],
                            in1=v[:, gi, C:2 * C],
                            s0=w(br, kk, 0, gi), s1=w(br, kk, 1, gi))
                    elif kind == 'dve3':
                        nc.scalar.activation(stc[:], v[:, gi, 0:C], AF.Copy,
                                             scale=w(br, kk, 0, gi))
                    else:
                        nc.vector.tensor_scalar_mul(stc[:], v[:, gi, 0:C],
                                                    w(br, kk, 0, gi))
                # wave 2: corners 2(+3) -> t2
                for slot, (br, kk, gi), kind in block:
                    stc, t2 = tiles[slot]
                    v = vtiles[(br, kk)]
                    if kind in ('axpt', 'axp'):
                        nc.vector._custom_dve(
                            _DUAL_AXPY, out=t2[:], in0=v[:, gi, 2 * C:3 * C],
                            in1=v[:, gi, 3 * C:4 * C],
                            s0=w(br, kk, 2, gi), s1=w(br, kk, 3, gi))
                    elif kind == 'dve3':
                        nc.scalar.activation(t2[:], v[:, gi, 2 * C:3 * C],
                                             AF.Copy, scale=w(br, kk, 2, gi))
                    else:
                        nc.vector.scalar_tensor_tensor(
                            stc[:], v[:, gi, C:2 * C], w(br, kk, 1, gi),
                            stc[:], ALU.mult, ALU.add)
                # previous block's sa evacs: PE transposes are long done
                for ev in pending_evacs:
                    ev()
                pending_evacs.clear()
                # wave 3: dve3: corner1 -> stc; dve2: corner2 -> stc
                for slot, (br, kk, gi), kind in block:
                    stc, t2 = tiles[slot]
                    v = vtiles[(br, kk)]
                    if kind == 'dve3':
                        nc.vector.scalar_tensor_tensor(
                            stc[:], v[:, gi, C:2 * C], w(br, kk, 1, gi),
                            stc[:], ALU.mult, ALU.add)
                    elif kind == 'dve2':
                        nc.vector.scalar_tensor_tensor(
                            stc[:], v[:, gi, 2 * C:3 * C], w(br, kk, 2, gi),
                            stc[:], ALU.mult, ALU.add)
                # wave 4: dve3: corner3 -> t2; dve2: corner3 -> stc
                for slot, (br, kk, gi), kind in block:
                    stc, t2 = tiles[slot]
                    v = vtiles[(br, kk)]
                    if kind == 'dve3':
                        nc.vector.scalar_tensor_tensor(
                            t2[:], v[:, gi, 3 * C:4 * C], w(br, kk, 3, gi),
                            t2[:], ALU.mult, ALU.add)
                    elif kind == 'dve2':
                        nc.vector.scalar_tensor_tensor(
                            stc[:], v[:, gi, 3 * C:4 * C], w(br, kk, 3, gi),
                            stc[:], ALU.mult, ALU.add)
                # wave 5: combine halves on DVE — one wide TT per all-axp
                # (br,kk), per-group TT/stt otherwise
                kind_of = {}
                tiles_of = {}
                for slot, g, kind in block:
                    kind_of[g] = kind
                    tiles_of[g] = tiles[slot]
                for br, kk in bks:
                    kinds = [kind_of[(br, kk, gi)] for gi in range(4)]
                    stc4, t24 = packs[(br, kk)]
                    if all(k in ('axp', 'axpt') for k in kinds):
                        nc.vector.tensor_tensor(stc4[:], stc4[:], t24[:],
                                                ALU.add)
                    else:
                        for gi in range(4):
                            if kinds[gi] in ('axp', 'axpt', 'dve3'):
                                stc, t2 = tiles_of[(br, kk, gi)]
                                nc.vector.tensor_tensor(stc[:], stc[:], t2[:],
                                                        ALU.add)
                # transposes: per completed (br,kk): gi/ch-blocks through PE
                # (identity matmul) into one [128,1024] f16 PSUM tile
                if mode == 'notr2':
                    return
                for br, kk in bks:
                    stc4, _ = packs[(br, kk)]
                    ptr = trppool.tile([128, 1024], F16, tag='ptr',
                                       name=f'ptr{r}_{br}_{kk}')
                    for ch in range(2):
                        for gi in range(4):
                            nc.tensor.transpose(
                                ptr[:, ch * 512 + gi * 128:
                                    ch * 512 + (gi + 1) * 128],
                                stc4[:, gi * C + ch * 128:
                                     gi * C + (ch + 1) * 128],
                                ident[:])
                    sa = sasm[(br, kk)]
                    if evac_act:
                        pending_evacs.append(
                            lambda sa=sa, ptr=ptr: nc.scalar.activation(
                                sa[:], ptr[:], AF.Copy))
                    else:
                        pending_evacs.append(
                            lambda sa=sa, ptr=ptr: nc.vector.tensor_copy(
                                sa[:], ptr[:]))

            def emit_einsum_br(br, sasm, pc_tiles, r):
                for o in range(2):
                    pc = catppool.tile([128, 512], F32, tag='pcat',
                                       name=f'pc{r}_{br}_{o}')
                    pc_tiles[(br, o)] = pc
                    for ck in range(CKT):
                        kk, ch = ck // 2, ck % 2
                        nc.tensor.matmul(
                            pc[:],
                            wp_t[br * CKT + ck][:, o * 128:(o + 1) * 128],
                            sasm[(br, kk)][:, ch * 512:(ch + 1) * 512],
                            start=(ck == 0), stop=(ck == CKT - 1))

            def emit_evac_br(br, pc_tiles, cat_tiles):
                for o in range(2):
                    ic = br * 2 + o
                    nc.scalar.activation(cat_tiles[ic][:], pc_tiles[(br, o)][:],
                                         AF.Copy)

            def make_tail(r, rep_r, cat_tiles, pc_tiles):
                """Round-r tail: head closure (evac br1 + 1x1 + h stores)
                plus a list of fine-grained 3x3 PE chunks, one per (nt, o),
                to interleave between the next round's blend blocks."""
                rep = rep_r // NR

                def head():
                    emit_evac_br(1, pc_tiles, cat_tiles)
                    for o in range(4):
                        ph = hppool.tile([128, 512], F32, tag='ph',
                                         name=f'ph{rep_r}_{o}')
                        for ic in range(4):
                            nc.tensor.matmul(
                                ph[:], wcc_t[ic][:, o * 128:(o + 1) * 128],
                                cat_tiles[ic][:], start=(ic == 0), stop=(ic == 3))
                        dst = h_t[o][:, :].copy()
                        pstep = dst.ap[0][0]
                        dst.offset = dst.offset + 68 + (r * RPR + 1) * H3 + 1
                        dst.ap = bass_rust.VecI64Pair(
                            [[pstep, 128], [H3, RPR], [1, Wd]])
                        nc.scalar.activation(dst, ph[:], AF.Identity,
                                             bias=bcc_t[:, o:o + 1])

                chunks = []
                if mode != 'noconv3':
                    ready = (r + 1) * RPR
                    for nt in range(NT3):
                        last = min(nt * RT3 + min(RT3, H3 - nt * RT3), H3 - 2)
                        prev_ready = r * RPR if r > 0 else -1
                        if last <= ready and not (last <= prev_ready):
                            for o in range(3):
                                chunks.append(
                                    lambda nt=nt, o=o: emit_3x3_o(nt, o, rep))
                return head, chunks

            # ---- main loop over rounds of 512 positions ----
            gidx = 0          # global gather counter: queue = gidx % 4 stays
            # aligned with Tile's DMASW lane round-robin (lane = gidx % 8),
            # so each sem lane only ever sees one SWDGE queue.
            pending_tail = None
            pending_pe = []
            for rep_r in range(reps * NR):
                r = rep_r % NR
                vtiles = {}
                for br in range(2):
                    for kk in range(KK):
                        col0 = (br * KK + kk) * (NR * 32) + r * 32
                        v = vpool.tile([128, 4, 4 * C], F16, tag='v',
                                       name=f'v{rep_r}_{br}_{kk}')
                        nc.gpsimd.dma_gather(
                            v[:], win, idx_t[:, col0:col0 + 32],
                            512, 512, 4 * C, elem_step=2 * C,
                            queue_num=gidx % 4)
                        gidx += 1
                        vtiles[(br, kk)] = v
                if mode == 'gatheronly':
                    for br in range(2):
                        for kk in range(KK):
                            col = ((r * 2 + br) * KK + kk) * 64
                            nc.sync.dma_start(
                                dbg_d[:, col:col + 64],
                                vtiles[(br, kk)][:, 0, 0:64])
                    continue
                if mode == 'noblend':
                    continue

                # group plan for this round
                groups = [(br, kk, gi)
                          for br in range(2) for kk in range(KK)
                          for gi in range(4)]
                sasm = {}
                if mode not in ('notr2',):
                    for br in range(2):
                        for kk in range(KK):
                            sasm[(br, kk)] = sapool.tile(
                                [128, 1024], F16, tag=f'sa{br}_{kk}',
                                name=f'sa{br}_{kk}_{rep_r}')

                cat_tiles = {}
                pc_tiles = {}
                for ic in range(4):
                    cat_tiles[ic] = catpool.tile([128, 512], F16, tag=f'cat{ic}',
                                                 name=f'cat{ic}_{rep_r}')

                # per-branch: blend blocks then einsum; pending tail from the
                # previous round flushes after the 2nd block of br0, and the
                # br0 cat evac lands after the 2nd block of br1.
                blk_count = 0
                pending_evacs = []
                for br in range(2):
                    gset = [g for g in groups if g[0] == br]
                    blocks = [gset[i:i + BLOCK] for i in range(0, len(gset), BLOCK)]
                    for bi, blk in enumerate(blocks):
                        block = []
                        for slot, g in enumerate(blk):
                            gidx72 = (g[0] * KK + g[1]) * 4 + g[2]
                            block.append((slot, g, pattern[gidx72]))
                        emit_blend_block(block, vtiles, sasm, r, pending_evacs)
                        blk_count += 1
                        if blk_count == 2 and pending_tail is not None:
                            pending_tail()
                            pending_tail = None
                        elif blk_count >= 3 and pending_pe:
                            pending_pe.pop(0)()
                        if br == 1 and bi == 1 and mode not in ('notr2',):
                            emit_evac_br(0, pc_tiles, cat_tiles)
                    if mode in ('notr2',):
                        continue
                    for ev in pending_evacs:
                        ev()
                    pending_evacs.clear()
                    emit_einsum_br(br, sasm, pc_tiles, r)

                if mode in ('notr2',):
                    continue
                pending_tail, new_chunks = make_tail(r, rep_r, cat_tiles,
                                                     pc_tiles)
                pending_pe.extend(new_chunks)

            if pending_tail is not None:
                pending_tail()
                pending_tail = None
            for chunk in pending_pe:
                chunk()
            pending_pe = []

    nc.compile()
    return nc


# ----------------------------------------------------------------- driver
_CACHE = {}


def _get_kernel(Hd, R):
    key = (Hd, R)
    if key not in _CACHE:
        _CACHE[key] = build_kernel(Hd, R)
    return _CACHE[key]


def global_pad(deform_map0, deform_map1, Hd):
    """Common pad P across the whole batch (all cores share one NEFF)."""
    P = 2
    for dms in (deform_map0, deform_map1):
        for b in range(dms.shape[0]):
            y0, x0, _ = _corner_geom(np.asarray(dms[b], np.float32), Hd)
            P = max(P, int(-y0.min()), int(y0.max() - 62),
                    int(-x0.min()), int(x0.max() - 62))
    return P


def prep_all(x, deform_map0, deform_map1, w_dc0, w_dc1, w_cc, b_cc, w_f, b_f):
    x = np.asarray(x, np.float32)
    Hd = x.shape[2]
    P = global_pad(np.asarray(deform_map0, np.float32),
                   np.asarray(deform_map1, np.float32), Hd)
    in_maps = []
    for b in range(x.shape[0]):
        m = host_prep(x[b], np.asarray(deform_map0[b], np.float32),
                      np.asarray(deform_map1[b], np.float32),
                      np.asarray(w_dc0, np.float32), np.asarray(w_dc1, np.float32),
                      np.asarray(w_cc, np.float32), np.asarray(b_cc, np.float32),
                      np.asarray(w_f, np.float32), np.asarray(b_f, np.float32),
                      Hd, P)
        in_maps.append(m)
    R = in_maps[0]['x2'].shape[0]
    return in_maps, Hd, R


def kernel(x, deform_map0, deform_map1, w_dc0, w_dc1, w_cc, b_cc, w_f, b_f):
    from concourse.bass_utils import run_bass_kernel_spmd
    in_maps, Hd, R = prep_all(x, deform_map0, deform_map1, w_dc0, w_dc1,
                              w_cc, b_cc, w_f, b_f)
    B = len(in_maps)
    nc = _get_kernel(Hd, R)
    res = run_bass_kernel_spmd(nc, in_maps, core_ids=list(range(B)))
    out = np.stack([res.results[b]['out'].reshape(NCLS, Hd, Hd) for b in range(B)])
    return out.astype(np.float32)
